# revision 20
# baseline (speedup 1.0000x reference)
"""Multi-head attention block on 8 TRN2 NeuronCores.

Problem: x[2,2048,768] -> qkv proj -> 12-head attention -> out proj.
Sharding: 24 (batch, head) pairs across 8 cores; core c handles batch
c//4 and heads 3*(c%4)..3*(c%4)+2. Each core computes its heads'
Q,K,V, attention, and a partial output projection; the host sums the
four per-batch partials and adds the bias terms.

Device notes:
  - All matmuls run in float32r (full-rate PE, ~tf32 precision). Two
    HW quirks shape the kernel: (1) the PE HAM clock-gate only
    sustains 2.4 GHz when the contraction dim drives all 128 array
    rows, so Q^T/K^T live in per-head [128, N] tiles whose bottom 64
    rows are zeros (zero rows annihilate the don't-care rows of the
    other operand) and the projection uses host-padded w_proj blocks;
    (2) f32r matmuls with free dim < 256 run at 1/4 rate, so the V
    projection's weights are host-padded from 192 to 256 columns.
  - Softmax: exp without max subtraction (logits ~ N(0,1));
    denominators come from a ones column appended to V (row 64 of the
    AV output); exp runs on 1024-wide tiles to amortize the ~260ns
    ScalarE overhead so TensorE stays the bottleneck.
  - Normalization: per (head, block), the denominator row is
    broadcast over 64 partitions with a K=1 matmul of a ones column,
    and one DVE divide writes the normalized attention output
    straight from PSUM. Everything overlaps the attention loop.
  - k-bias is dropped (softmax shift invariance along keys); v-bias
    and proj-bias fold in on the host: out += b_proj + b_v @ w_proj.
  - q-bias and the 1/sqrt(D) scale fuse into the Q^T PSUM->SBUF copy.
  - Input DMAs are ordered so the first QKV matmul group's operands
    (w_q + the first x^T column block) land first.
"""

import os
import sys

for _p in ("/opt/trn_rl_repo", "/opt/pypackages"):
    if _p not in sys.path:
        sys.path.append(_p)

import numpy as np

B, N, C = 2, 2048, 768
H, D = 12, 64
HPC = 3                    # heads per core
J = HPC * D                # 192: per-core head-dim rows
JV = 256                   # v-projection free dim, padded up from J
NCORES = 8
NBLK = 1024                # query-block width (one exp per [128, NBLK])
NB = N // NBLK             # 2
MC = N // 128              # 16 key chunks
KC = C // 128              # 6 contraction chunks for projections

_cache = {}
LAST_RESULTS = None


def _build():
    import concourse.mybir as mybir
    import concourse.tile as tile
    from concourse import bacc

    f32 = mybir.dt.float32
    f32r = mybir.dt.float32r
    Exp = mybir.ActivationFunctionType.Exp
    mult = mybir.AluOpType.mult
    add = mybir.AluOpType.add
    div = mybir.AluOpType.divide

    nc = bacc.Bacc("TRN2", target_bir_lowering=False, debug=False,
                   num_devices=NCORES)

    xt_d = nc.declare_dram_parameter("xt", [C, N], f32r, isOutput=False)
    wq_d = nc.declare_dram_parameter("wq", [C, J], f32r, isOutput=False)
    wk_d = nc.declare_dram_parameter("wk", [C, J], f32r, isOutput=False)
    wv_d = nc.declare_dram_parameter("wv", [C, JV], f32r, isOutput=False)
    bq_d = nc.declare_dram_parameter("bq", [J, 1], f32, isOutput=False)
    ones_d = nc.declare_dram_parameter("ones", [128, 67], f32r,
                                       isOutput=False)
    zeros_d = nc.declare_dram_parameter("zeros", [64, N], f32r,
                                        isOutput=False)
    # per-head padded proj weights: 3 blocks of [128, C], bottom 64
    # rows of each block are zero
    wp_d = nc.declare_dram_parameter("wp", [HPC * 128, C], f32r,
                                     isOutput=False)
    out_d = nc.declare_dram_parameter("out", [N, C], f32, isOutput=True)

    with tile.TileContext(nc) as tc:
        with (
            tc.tile_pool(name="persist", bufs=1) as pp,
            tc.tile_pool(name="osb", bufs=4) as posb,
        ):
            bqt = [pp.tile([64, 1], f32, tag=f"bq{h}", name=f"bq{h}")
                   for h in range(HPC)]
            for h in range(HPC):
                nc.sync.dma_start(bqt[h][:], bq_d[64 * h:64 * (h + 1), :])
            wq = [pp.tile([128, J], f32r, tag=f"wq{i}", name=f"wq{i}")
                  for i in range(KC)]
            wk = [pp.tile([128, J], f32r, tag=f"wk{i}", name=f"wk{i}")
                  for i in range(KC)]
            wv = [pp.tile([128, JV], f32r, tag=f"wv{i}", name=f"wv{i}")
                  for i in range(KC)]
            for i in range(KC):
                nc.sync.dma_start(wq[i][:],
                                  wq_d[128 * i:128 * (i + 1), :])
            for i in range(KC):
                nc.sync.dma_start(wk[i][:],
                                  wk_d[128 * i:128 * (i + 1), :])

            # per-head padded Q^T/K^T: rows 0:64 data, rows 64:128 zero
            qh = [pp.tile([128, N], f32r, tag=f"qh{h}", name=f"qh{h}")
                  for h in range(HPC)]
            kh = [pp.tile([128, N], f32r, tag=f"kh{h}", name=f"kh{h}")
                  for h in range(HPC)]
            # V with a ones column per head: [128, 3*65]
            vx = [pp.tile([128, HPC * 65], f32r, tag=f"vx{m}", name=f"vx{m}")
                  for m in range(MC)]
            ones_t = pp.tile([128, 67], f32r, tag="ones_t", name="ones_t")
            # denominator rows (f32: feeds the approx reciprocal)
            sums = [pp.tile([1, N], f32, tag=f"sums{h}", name=f"sums{h}")
                    for h in range(HPC)]
            wp = [pp.tile([128, C], f32r, tag=f"wp{h}", name=f"wp{h}")
                  for h in range(HPC)]

            # ---- Phase 1: Q^T, K^T (d-major, padded) and V ----
            with (
                tc.tile_pool(name="xtp", bufs=1) as pxt,
                tc.tile_pool(name="ps1", bufs=2, space="PSUM") as ps1,
            ):
                xt = [pxt.tile([128, N], f32r, tag=f"xt{i}", name=f"xt{i}")
                      for i in range(KC)]
                zt = pp.tile([64, N], f32r, tag="zt", name="zt")
                for nb in range(4):
                    nsl = slice(512 * nb, 512 * (nb + 1))
                    for i in range(KC):
                        nc.sync.dma_start(
                            xt[i][:, nsl],
                            xt_d[128 * i:128 * (i + 1), nsl])
                    if nb == 0:
                        for i in range(KC):
                            nc.sync.dma_start(wv[i][:],
                                              wv_d[128 * i:128 * (i + 1), :])
                        nc.gpsimd.dma_start(zt[:], zeros_d[:, :])
                        for h in range(HPC):
                            nc.gpsimd.dma_start(qh[h][64:128, :], zt[:])
                            nc.gpsimd.dma_start(kh[h][64:128, :], zt[:])
                nc.sync.dma_start(ones_t[:], ones_d[:, :])
                for m in range(MC):
                    on = vx[m].rearrange("p (h e) -> p h e", e=65)[:, :, 64:65]
                    nc.vector.tensor_copy(
                        on, ones_t[:, 64:67].rearrange("p (h e) -> p h e",
                                                       e=1))
                for h in range(HPC):
                    nc.sync.dma_start(wp[h][:],
                                      wp_d[128 * h:128 * (h + 1), :])

                for nb in range(4):
                    nsl = slice(512 * nb, 512 * (nb + 1))
                    for heads, wsl in [((0, 1), slice(0, 128)),
                                       ((2,), slice(128, 192))]:
                        pn = 64 * len(heads)
                        ps = ps1.tile([128, 512], f32, tag="qk", bufs=4,
                                      name="ps_q")
                        for k in range(KC):
                            nc.tensor.matmul(
                                ps[:pn, :], wq[k][:, wsl], xt[k][:, nsl],
                                start=(k == 0), stop=(k == KC - 1))
                        for j, h in enumerate(heads):
                            nc.vector.tensor_scalar(
                                qh[h][0:64, nsl],
                                ps[64 * j:64 * (j + 1), :], 0.125,
                                bqt[h][:], mult, add)
                        ps = ps1.tile([128, 512], f32, tag="qk", bufs=4,
                                      name="ps_k")
                        for k in range(KC):
                            nc.tensor.matmul(
                                ps[:pn, :], wk[k][:, wsl], xt[k][:, nsl],
                                start=(k == 0), stop=(k == KC - 1))
                        for j, h in enumerate(heads):
                            nc.vector.tensor_copy(
                                kh[h][0:64, nsl],
                                ps[64 * j:64 * (j + 1), :])
                    for m in range(4 * nb, 4 * nb + 4):
                        msl = slice(128 * m, 128 * (m + 1))
                        ps = ps1.tile([128, JV], f32, tag="v", bufs=3,
                                      name="ps_v")
                        for k in range(KC):
                            nc.tensor.matmul(ps[:], xt[k][:, msl], wv[k][:],
                                             start=(k == 0),
                                             stop=(k == KC - 1))
                        vdst = vx[m].rearrange("p (h e) -> p h e",
                                               e=65)[:, :, 0:64]
                        nc.vector.tensor_copy(
                            vdst,
                            ps[:, 0:J].rearrange("p (h e) -> p h e", e=64))

            # attention-phase tiles reuse the x^T address range
            with (
                tc.tile_pool(name="attn", bufs=1) as pat,
                tc.tile_pool(name="etile", bufs=4) as pe,
                tc.tile_pool(name="bcsb", bufs=2) as pbc,
            ):
                # normalized, zero-padded attention outputs
                ah2 = [pat.tile([128, N], f32r, tag=f"ah2{h}",
                                name=f"ah2{h}") for h in range(HPC)]
                for h in range(HPC):
                    nc.gpsimd.dma_start(ah2[h][64:128, :], zt[:])

                # ---- Phase 2: attention, one continuous pipeline ----
                # The s -> exp -> AV chain runs in a single global pend
                # queue across all (head, block) pairs so the PE/ACT
                # pipeline never drains at block boundaries.
                with tc.tile_pool(name="ps2", bufs=1, space="PSUM") as ps2:
                    pend = []

                    def flush_one():
                        avh, h, nb, mm, ee = pend.pop(0)
                        vsl = slice(65 * h, 65 * (h + 1))
                        for i in range(NBLK // 512):
                            nc.tensor.matmul(
                                avh[i][:], vx[mm][:, vsl],
                                ee[:, 512 * i:512 * (i + 1)],
                                start=(mm == 0), stop=(mm == MC - 1))
                        if mm != MC - 1:
                            return
                        # block complete: normalize -- broadcast the
                        # denominator row, approx-reciprocal, multiply;
                        # all off the critical PE path
                        nsl = slice(NBLK * nb, NBLK * (nb + 1))
                        for i in range(NBLK // 512):
                            hf = slice(NBLK * nb + 512 * i,
                                       NBLK * nb + 512 * (i + 1))
                            nc.vector.tensor_copy(sums[h][:, hf],
                                                  avh[i][64:65, :])
                        bcs = pbc.tile([64, NBLK], f32, tag="bcs",
                                       name="bcs")
                        nc.gpsimd.partition_broadcast(
                            bcs[:], sums[h][:, nsl])
                        rec = pbc.tile([64, NBLK], f32, tag="rec",
                                       name="rec")
                        nc.vector.reciprocal_approx_fast(rec[:], bcs[:])
                        for i in range(NBLK // 512):
                            hf = slice(NBLK * nb + 512 * i,
                                       NBLK * nb + 512 * (i + 1))
                            nc.vector.tensor_mul(
                                ah2[h][0:64, hf], avh[i][0:64, :],
                                rec[:, 512 * i:512 * (i + 1)])

                    for h in range(HPC):
                        for nb in range(NB):
                            avh = [ps2.tile([65, 512], f32, tag=f"av{i}",
                                            bufs=2, name=f"ps_av{i}")
                                   for i in range(NBLK // 512)]
                            for m in range(MC):
                                msl = slice(128 * m, 128 * (m + 1))
                                s = ps2.tile([128, NBLK], f32, tag="s",
                                             bufs=2, name="ps_s")
                                for i in range(NBLK // 512):
                                    nc.tensor.matmul(
                                        s[:, 512 * i:512 * (i + 1)],
                                        kh[h][:, msl],
                                        qh[h][:, NBLK * nb + 512 * i:
                                              NBLK * nb + 512 * (i + 1)])
                                e = pe.tile([128, NBLK], f32r, tag="e",
                                            name="e")
                                nc.scalar.activation(e[:], s[:], Exp)
                                pend.append((avh, h, nb, m, e))
                                # keep PE ~2 steps ahead of ACT
                                if len(pend) > 2:
                                    flush_one()
                    while pend:
                        flush_one()

                # ---- Phase 3: output projection over padded heads ----
                with tc.tile_pool(name="ps3", bufs=4, space="PSUM") as ps3:
                    FCH = [(0, 512), (512, 256)]
                    for m in range(MC):
                        msl = slice(128 * m, 128 * (m + 1))
                        ps = ps3.tile([128, C], f32, tag="pj", name="ps_pj")
                        for f0, fn in FCH:
                            for h in range(HPC):
                                nc.tensor.matmul(
                                    ps[:, f0:f0 + fn], ah2[h][:, msl],
                                    wp[h][:, f0:f0 + fn],
                                    start=(h == 0), stop=(h == HPC - 1))
                        o3 = posb.tile([128, C], f32, tag="o3", name="o3")
                        nc.vector.tensor_copy(o3[:], ps[:])
                        if m % 2 == 0:
                            nc.sync.dma_start(out_d[msl, :], o3[:])
                        else:
                            nc.gpsimd.dma_start(out_d[msl, :], o3[:])

    nc.compile()
    return nc


def kernel(x, w_qkv, b_qkv, w_proj, b_proj):
    from concourse.bass_utils import run_bass_kernel_spmd

    global LAST_RESULTS
    if "nc" not in _cache:
        _cache["nc"] = _build()
    nc = _cache["nc"]

    x = np.asarray(x, dtype=np.float32)
    w_qkv = np.asarray(w_qkv, dtype=np.float32)
    b_qkv = np.asarray(b_qkv, dtype=np.float32)
    w_proj = np.asarray(w_proj, dtype=np.float32)
    b_proj = np.asarray(b_proj, dtype=np.float32)

    in_maps = []
    for c in range(NCORES):
        b = c // 4
        h0 = HPC * (c % 4)
        cs = slice(64 * h0, 64 * (h0 + HPC))
        ks = slice(C + 64 * h0, C + 64 * (h0 + HPC))
        wv_pad = np.zeros((C, JV), dtype=np.float32)
        wv_pad[:, 0:J] = w_qkv[:, 2 * C + 64 * h0:2 * C + 64 * (h0 + HPC)]
        wp_pad = np.zeros((HPC * 128, C), dtype=np.float32)
        for h in range(HPC):
            wp_pad[128 * h:128 * h + 64] = \
                w_proj[64 * (h0 + h):64 * (h0 + h + 1), :]
        in_maps.append({
            "xt": np.ascontiguousarray(x[b].T),
            "wq": np.ascontiguousarray(w_qkv[:, cs]),
            "wk": np.ascontiguousarray(w_qkv[:, ks]),
            "wv": wv_pad,
            "bq": np.ascontiguousarray(
                (b_qkv[cs] * 0.125).reshape(J, 1)),
            "ones": np.ones((128, 67), dtype=np.float32),
            "zeros": np.zeros((64, N), dtype=np.float32),
            "wp": wp_pad,
        })

    res = run_bass_kernel_spmd(nc, in_maps, core_ids=list(range(NCORES)))
    LAST_RESULTS = res

    out = np.zeros((B, N, C), dtype=np.float32)
    for c in range(NCORES):
        out[c // 4] += res.results[c]["out"]
    out += b_proj + b_qkv[2 * C:] @ w_proj
    return out
